# revision 1
# baseline (speedup 1.0000x reference)
"""Distributed multi-head attention (L=4096, D=2048, H=16, d=128) on 8 TRN2 cores.

Strategy: tensor-parallel over heads (2 heads per core) for QKV projections +
attention, then AllToAll (3 pieces, first two overlapped with the attention
tail) to switch to sequence-parallel for the output projection. Each core
returns 512 rows of the final output; the host reassembles.

Per-core dataflow (matmuls in bf16, f32 PSUM accumulation):
  1. QT/KT = Wq/Wk_shard.T @ x.T   ([d, L] layout, head dim on partitions)
     V     = x @ Wv_shard          ([L, f] layout, ones columns interleaved)
     RoPE applied to QT/KT in [d, L] layout with host-prepared C/S tables
     (the 1/sqrt(d) scale is folded into the exp activation).
  2. Per head: scoresT[k, q] = KT_tile.T @ QT_block -> exp on ScalarE -> PT
     bf16; out[q, d|sum] = PT.T @ [V|1] (softmax denominator rides along as a
     129th matmul column); rows normalized by its reciprocal, then transposed
     on the TensorE so the AllToAll lands in [d_concat, q] (lhsT) layout.
  3. After each A2A piece the core holds AT_s[2048, q_s] for its own output
     rows; plain chunk loads feed out = A @ Wo.
"""

import os
import sys
import types

import numpy as np
import ml_dtypes

import concourse.bass as bass
import concourse.mybir as mybir
import concourse.tile as tile
from concourse import bacc
from concourse.bass_utils import run_bass_kernel_spmd
from concourse.masks import make_identity
from concourse.tile_rust import add_dep_helper

BF16 = mybir.dt.bfloat16
F32 = mybir.dt.float32
nbf16 = ml_dtypes.bfloat16

N_CORES = 8
L = 4096
D = 2048
HPC = 2  # heads per core
HD = 128  # head dim
FC = HPC * HD  # 256: per-core projection width
KCH = D // 128  # 16 contraction chunks
SCALE = 1.0 / float(np.sqrt(HD))
QB = 512  # attention q block
NKK = L // 128  # 32 key tiles

# A2A splits: (q0, qlen, block_q). Output rows of core c:
#   s0 -> global [c*256, +256),  s1 -> [2048+c*128, +128),  s2 -> [3072+c*128, +128)
SPLITS = [(0, 2048, 256), (2048, 1024, 128), (3072, 1024, 128)]
OUT_ROW0 = [0, 256, 384]

# module-level knobs (test.py pokes these)
TRACE = False
LAST_RESULTS = None
_CACHED = {}


def _patch_walrus_flags():
    """Enable the LDWEIGHTS fast path (fast weight load) in walrus."""
    from concourse import bass_utils as _bu

    if getattr(_bu, "_ldw_patched", False):
        return
    _orig = _bu.run_command

    def _patched(cmd, **kw):
        cmd = [
            c
            for c in cmd
        ]
        return _orig(cmd, **kw)

    _bu.run_command = _patched
    _bu._ldw_patched = True


def _install_ntff_hook():
    """Enable NTFF profiling under axon (the container lacks antenv.axon_hooks)."""
    try:
        if "antenv.axon_hooks" not in sys.modules:
            mod = types.ModuleType("antenv.axon_hooks")
            _hook = [None]
            mod.set_axon_ntff_profile_hook = lambda h: _hook.__setitem__(0, h)
            mod.get_axon_ntff_profile_hook = lambda: _hook[0]
            sys.modules["antenv.axon_hooks"] = mod
            import antenv

            antenv.axon_hooks = mod
        from antenv.axon_hooks import set_axon_ntff_profile_hook
        from trn_agent_boot.trn_boot import _ntff_profile_via_ctypes

        set_axon_ntff_profile_hook(_ntff_profile_via_ctypes("/opt/axon/libaxon_pjrt.so"))
        from concourse import bass_utils

        bass_utils.upload_artifacts = lambda tmpdir: tmpdir
    except Exception:
        pass


def build_nc():
    nc = bacc.Bacc(None, target_bir_lowering=False, num_devices=N_CORES)

    xT_ext = nc.declare_dram_parameter("xT", [D, L], BF16, isOutput=False)
    wq_ext = nc.declare_dram_parameter("wq", [D, FC], BF16, isOutput=False)
    wk_ext = nc.declare_dram_parameter("wk", [D, FC], BF16, isOutput=False)
    wv_ext = nc.declare_dram_parameter("wv", [D, FC], BF16, isOutput=False)
    wo_ext = nc.declare_dram_parameter("wo", [D, D], BF16, isOutput=False)
    ctab_ext = nc.declare_dram_parameter("ctab", [128, L], BF16, isOutput=False)
    stab_ext = nc.declare_dram_parameter("stab", [128, L], BF16, isOutput=False)
    out_ext = nc.declare_dram_parameter("out", [512, D], F32, isOutput=True)

    # A2A bounces in [d_concat-block, q] layout: rows = 8 blocks x (2 heads x 128 d)
    a2a_in = [
        nc.dram_tensor(f"a2a_in{s}", [8 * FC, bq], BF16)
        for s, (_, _, bq) in enumerate(SPLITS)
    ]
    a2a_out = [
        nc.dram_tensor(f"a2a_out{s}", [8 * FC, bq], BF16)
        for s, (_, _, bq) in enumerate(SPLITS)
    ]

    sync_in = nc.dram_tensor("sync_in", [8, 64], BF16)
    sync_out = nc.dram_tensor("sync_out", [8, 64], BF16)

    with tile.TileContext(nc) as tc:
        with tc.tile_pool(name="persist", bufs=1) as persist:
            # barrier-warming collective: absorbs per-core start skew so the
            # first real AllToAll doesn't pay it
            nc.gpsimd.collective_compute(
                "AllToAll",
                mybir.AluOpType.bypass,
                replica_groups=[list(range(N_CORES))],
                ins=[sync_in.ap().opt()],
                outs=[sync_out.ap().opt()],
            )
            # persistent tiles (no DMA yet; emission order sets DMA priority)
            qt = persist.tile([128, HPC * L], BF16, tag="qt")
            kt = persist.tile([128, HPC * L], BF16, tag="kt")
            va = persist.tile([128, (L // 128) * 260], BF16, tag="va")
            wo_sb = persist.tile([128, KCH * D], BF16, tag="wo")
            warm = persist.tile([128, 8], BF16, tag="warm")

            QL = 1024  # L columns per xT load round
            with (
                tc.tile_pool(name="p1sb", bufs=1) as p1,
                tc.tile_pool(name="p1ps", bufs=1, space="PSUM") as p1ps,
            ):
                # critical-path loads first: wq, wk, xT quarter 0
                ctab = p1.tile([128, L], BF16, tag="ctab")
                stab = p1.tile([128, L], BF16, tag="stab")
                wq_sb = p1.tile([128, KCH * FC], BF16, tag="wq")
                wk_sb = p1.tile([128, KCH * FC], BF16, tag="wk")
                wv_sb = p1.tile([128, KCH * FC], BF16, tag="wv")
                for w_sb, w_ext in ((wq_sb, wq_ext), (wk_sb, wk_ext)):
                    nc.gpsimd.dma_start(
                        w_sb[:].rearrange("p (k f) -> p k f", k=KCH),
                        w_ext[:].rearrange("(k p) f -> p k f", p=128),
                    )
                xts_q0 = []
                for kc in range(KCH):
                    xt_t = p1.tile([128, QL], BF16, tag="xt", bufs=18)
                    eng = (nc.gpsimd, nc.sync)[kc % 2]
                    eng.dma_start(xt_t[:], xT_ext[kc * 128 : (kc + 1) * 128, 0:QL])
                    xts_q0.append(xt_t)
                nc.gpsimd.dma_start(ctab[:], ctab_ext[:])
                nc.gpsimd.dma_start(stab[:], stab_ext[:])
                nc.gpsimd.dma_start(
                    wv_sb[:].rearrange("p (k f) -> p k f", k=KCH),
                    wv_ext[:].rearrange("(k p) f -> p k f", p=128),
                )
                nc.gpsimd.memset(va[:], 1.0)
                nc.scalar.activation(
                    warm[:], ctab[:, 0:8], mybir.ActivationFunctionType.Exp
                )
                nc.gpsimd.dma_start(
                    wo_sb[:].rearrange("p (k f) -> p k f", k=KCH),
                    wo_ext[:].rearrange("(k p) f -> p k f", p=128),
                )

                # ---------------- Phase 1: QKV projections + RoPE ------------
                for quarter in range(L // QL):
                    l0 = quarter * QL
                    if quarter == 0:
                        xts = xts_q0
                    else:
                        xts = []
                        for kc in range(KCH):
                            xt_t = p1.tile([128, QL], BF16, tag="xt", bufs=18)
                            nc.gpsimd.dma_start(
                                xt_t[:], xT_ext[kc * 128 : (kc + 1) * 128, l0 : l0 + QL]
                            )
                            xts.append(xt_t)
                    # Q and K projections (transposed layout) + rope
                    for (w_sb, dst) in ((wq_sb, qt), (wk_sb, kt)):
                        for h in range(HPC):
                            for lb in range(QL // 512):
                                ps = p1ps.tile([128, 512], F32, tag="pj", bufs=4)
                                for kc in range(KCH):
                                    nc.tensor.matmul(
                                        ps[:],
                                        w_sb[:, kc * FC + h * HD : kc * FC + (h + 1) * HD],
                                        xts[kc][:, lb * 512 : (lb + 1) * 512],
                                        start=(kc == 0),
                                        stop=(kc == KCH - 1),
                                    )
                                lsl = slice(l0 + lb * 512, l0 + (lb + 1) * 512)
                                tmp = p1.tile([128, 512], BF16, tag="tmp", bufs=4)
                                nc.scalar.copy(tmp[:], ps[:])
                                rot = p1.tile([128, 512], BF16, tag="rot", bufs=4)
                                for (a, b) in ((0, 64), (32, 96), (64, 0), (96, 32)):
                                    nc.vector.tensor_copy(
                                        rot[a : a + 32, :], tmp[b : b + 32, :]
                                    )
                                t1 = p1.tile([128, 512], BF16, tag="t1", bufs=4)
                                nc.vector.tensor_mul(t1[:], tmp[:], ctab[:, lsl])
                                t2 = p1.tile([128, 512], BF16, tag="t2", bufs=4)
                                nc.vector.tensor_mul(t2[:], rot[:], stab[:, lsl])
                                dsl = slice(
                                    h * L + l0 + lb * 512, h * L + l0 + (lb + 1) * 512
                                )
                                nc.vector.tensor_add(dst[:, dsl], t1[:], t2[:])
                    # V projection (natural layout), strided copy into va
                    for lt in range(QL // 128):
                        psv = p1ps.tile([128, FC], F32, tag="pv", bufs=3)
                        for kc in range(KCH):
                            nc.tensor.matmul(
                                psv[:],
                                xts[kc][:, lt * 128 : (lt + 1) * 128],
                                wv_sb[:, kc * FC : (kc + 1) * FC],
                                start=(kc == 0),
                                stop=(kc == KCH - 1),
                            )
                        gt = quarter * (QL // 128) + lt  # global L tile 0..31
                        dst = va[:, gt * 260 : (gt + 1) * 260].rearrange(
                            "p (g j) -> p g j", g=2
                        )[:, :, 0:128]
                        nc.vector.tensor_copy(
                            dst, psv[:].rearrange("p (g j) -> p g j", g=2)
                        )

            # -------- Phase 2: attention windows + overlapped A2A/Wo ---------
            windows = [(qb, h) for qb in range(L // QB) for h in range(HPC)]
            with (
                tc.tile_pool(name="p2sb", bufs=1) as p2,
                tc.tile_pool(name="p2ps", bufs=1, space="PSUM") as p2ps,
                tc.tile_pool(name="p3sb", bufs=1) as p3,
                tc.tile_pool(name="p3ps", bufs=1, space="PSUM") as p3ps,
            ):
                pt_store = {}

                def emit_epilogue(w, qs, av):
                    qb, h = windows[w]
                    rec = p2.tile([128, 1], F32, tag="rec", bufs=12)
                    nc.vector.reciprocal(rec[:], av[:, 128:129])
                    osb = p2.tile([128, 128], BF16, tag="osb", bufs=12)
                    nc.vector.tensor_scalar_mul(osb[:], av[:, 0:128], rec[:])
                    # xbar-transpose on the sync DMA queue so the A2A
                    # carries [d, q]-layout tiles
                    ot = p2.tile([128, 128], BF16, tag="ot", bufs=12)
                    nc.sync.dma_start_transpose(ot[:], osb[:])
                    t = qb * (QB // 128) + qs  # global q tile 0..31
                    qg = t * 128
                    s = next(
                        i for i, (q0, ql, _) in enumerate(SPLITS) if q0 <= qg < q0 + ql
                    )
                    q0, _, bq = SPLITS[s]
                    tt = (qg - q0) // 128
                    tpb = bq // 128
                    j, co = tt // tpb, (tt % tpb) * 128
                    nc.gpsimd.dma_start(
                        a2a_in[s][j * FC + h * HD : j * FC + (h + 1) * HD, co : co + 128],
                        ot[:],
                    )

                def emit_window(w):
                    # scores+exp for window w interleaved with AV for w-1, so
                    # the PE never idles waiting on the exp pipeline
                    prev = pt_store.pop(w - 1, None)
                    pts = []
                    av = None
                    last_av_mm = None
                    for p in range(NKK // 2):  # 16 steps
                        if w < len(windows):
                            qb, h = windows[w]
                            sc = p2ps.tile([128, 1024], F32, tag="sc", bufs=2)
                            for half in range(2):
                                kk = p * 2 + half
                                nc.tensor.matmul(
                                    sc[:, half * 512 : (half + 1) * 512],
                                    kt[:, h * L + kk * 128 : h * L + (kk + 1) * 128],
                                    qt[:, h * L + qb * QB : h * L + (qb + 1) * QB],
                                )
                            pt = p2.tile([128, 1024], BF16, tag="pt", bufs=32)
                            nc.scalar.activation(
                                pt[:], sc[:], mybir.ActivationFunctionType.Exp, scale=SCALE
                            )
                            pts.append(pt)
                        if prev is not None:
                            pqb, ph = windows[w - 1]
                            qs = p // 4
                            if p % 4 == 0:
                                av = p2ps.tile([128, 129], F32, tag="av", bufs=3)
                            for kk in range((p % 4) * 8, (p % 4) * 8 + 8):
                                last_av_mm = nc.tensor.matmul(
                                    av[:],
                                    prev[kk // 2][
                                        :,
                                        (kk % 2) * 512 + qs * 128 : (kk % 2) * 512 + (qs + 1) * 128,
                                    ],
                                    va[:, kk * 260 + ph * 130 : kk * 260 + ph * 130 + 129],
                                    start=(kk == 0),
                                    stop=(kk == NKK - 1),
                                    skip_group_check=True,
                                )
                            if p % 4 == 3:
                                emit_epilogue(w - 1, qs, av)
                    if pts:
                        pt_store[w] = pts
                    return last_av_mm

                ats = {}

                def emit_wo_comm(s):
                    nc.gpsimd.collective_compute(
                        "AllToAll",
                        mybir.AluOpType.bypass,
                        replica_groups=[list(range(N_CORES))],
                        ins=[a2a_in[s].ap().opt()],
                        outs=[a2a_out[s].ap().opt()],
                    )

                def emit_at_load(s, after=None):
                    # deferred so the sync queue isn't head-of-line blocked
                    # on the collective while epilogue transposes queue up
                    bq = SPLITS[s][2]
                    at = p3.tile([128, KCH * 256], BF16, tag="at", bufs=2)
                    ld = nc.sync.dma_start(
                        at[:, 0 : KCH * bq].rearrange("p (k q) -> p k q", k=KCH),
                        a2a_out[s][:].rearrange("(k p) q -> p k q", p=128),
                    )
                    if after is not None:
                        add_dep_helper(
                            ld.ins,
                            after.ins,
                            sync=False,
                            reason="keep AT load behind the attention window",
                        )
                    ats[s] = at

                def emit_wo_chunk(s, g, after=None):
                    bq = SPLITS[s][2]
                    rt, fb = g // 4, g % 4
                    po = p3ps.tile([128, 512], F32, tag="po", bufs=1)
                    for kc in range(KCH):
                        mm = nc.tensor.matmul(
                            po[:],
                            ats[s][:, kc * bq + rt * 128 : kc * bq + (rt + 1) * 128],
                            wo_sb[:, kc * D + fb * 512 : kc * D + (fb + 1) * 512],
                            start=(kc == 0),
                            stop=(kc == KCH - 1),
                            skip_group_check=True,
                        )
                        if kc == 0 and after is not None:
                            add_dep_helper(
                                mm.ins,
                                after.ins,
                                sync=False,
                                reason="keep Wo chunk behind the attention window",
                            )
                    ob = p3.tile([128, 512], F32, tag="ob", bufs=2)
                    # DVE, not ACT: ACT is saturated with exp during the windows,
                    # and the single-buffered po bank recycles through this copy
                    nc.vector.tensor_copy(ob[:], po[:])
                    r0 = OUT_ROW0[s] + rt * 128
                    nc.sync.dma_start(
                        out_ext[r0 : r0 + 128, fb * 512 : (fb + 1) * 512], ob[:]
                    )

                for w in range(len(windows) + 1):
                    la = emit_window(w)
                    if w == 8:
                        # split-0 tiles (q < 2048, both heads) stored after
                        # emit_av(7) inside this same iteration
                        emit_wo_comm(0)
                    if w == 12:
                        emit_at_load(0, after=la)
                        # split-1 tiles (2048 <= q < 3072) stored after
                        # emit_av(11) inside this same iteration
                        emit_wo_comm(1)
                    if w == 14:
                        emit_at_load(1, after=la)
                    if 13 <= w <= 16:
                        emit_wo_chunk(0, 2 * (w - 13), after=la)
                        emit_wo_chunk(0, 2 * (w - 13) + 1, after=la)
                    if w == 15:
                        emit_wo_chunk(1, 0, after=la)
                    if w == 16:
                        emit_wo_chunk(1, 1, after=la)
                emit_wo_chunk(1, 2)
                emit_wo_chunk(1, 3)
                emit_wo_comm(2)
                emit_at_load(2)
                for g in range(4):
                    emit_wo_chunk(2, g)

    nc.compile()
    return nc


def _host_prep(x, Wq, Wk, Wv, Wo, sin, cos):
    xT = np.ascontiguousarray(x.T).astype(nbf16)
    wo_b = np.ascontiguousarray(Wo).astype(nbf16)
    c64 = cos.reshape(L, 64)
    s64 = sin.reshape(L, 64)
    ctab = np.ascontiguousarray(np.concatenate([c64, c64], axis=1).T).astype(nbf16)
    stab = np.ascontiguousarray(np.concatenate([-s64, s64], axis=1).T).astype(nbf16)
    in_maps = []
    for c in range(N_CORES):
        sl = slice(c * FC, (c + 1) * FC)
        in_maps.append(
            {
                "xT": xT,
                "wq": np.ascontiguousarray(Wq[:, sl]).astype(nbf16),
                "wk": np.ascontiguousarray(Wk[:, sl]).astype(nbf16),
                "wv": np.ascontiguousarray(Wv[:, sl]).astype(nbf16),
                "wo": wo_b,
                "ctab": ctab,
                "stab": stab,
            }
        )
    return in_maps


def kernel(x, Wq, Wk, Wv, Wo, sin, cos):
    global LAST_RESULTS
    x, Wq, Wk, Wv, Wo = (np.asarray(a, np.float32) for a in (x, Wq, Wk, Wv, Wo))
    sin, cos = np.asarray(sin, np.float32), np.asarray(cos, np.float32)

    _patch_walrus_flags()
    if TRACE:
        _install_ntff_hook()
        os.environ["BASS_TRACE"] = "1"

    if "nc" not in _CACHED:
        _CACHED["nc"] = build_nc()
    nc = _CACHED["nc"]

    in_maps = _host_prep(x, Wq, Wk, Wv, Wo, sin, cos)
    trace_cores = list(range(N_CORES)) if os.environ.get("ALL_CORES") else None
    res = run_bass_kernel_spmd(
        nc, in_maps, core_ids=list(range(N_CORES)), trace=TRACE, trace_cores=trace_cores
    )
    LAST_RESULTS = res

    out = np.empty((L, D), np.float32)
    for c in range(N_CORES):
        oc = res.results[c]["out"]
        out[c * 256 : (c + 1) * 256] = oc[0:256]
        out[2048 + c * 128 : 2048 + (c + 1) * 128] = oc[256:384]
        out[3072 + c * 128 : 3072 + (c + 1) * 128] = oc[384:512]
    return out



# revision 5
# speedup vs baseline: 1.0536x; 1.0536x over previous
"""Distributed multi-head attention (L=4096, D=2048, H=16, d=128) on 8 TRN2 cores.

Strategy: tensor-parallel over heads (2 heads per core) for QKV projections +
attention, then AllToAll (3 pieces, first two overlapped with the attention
tail) to switch to sequence-parallel for the output projection. Each core
returns 512 rows of the final output; the host reassembles.

Per-core dataflow (matmuls in bf16, f32 PSUM accumulation):
  1. QT/KT = Wq/Wk_shard.T @ x.T   ([d, L] layout, head dim on partitions)
     V     = x @ Wv_shard          ([L, f] layout, ones columns interleaved)
     RoPE applied to QT/KT in [d, L] layout with host-prepared C/S tables
     (the 1/sqrt(d) scale is folded into the exp activation).
  2. Per head: scoresT[k, q] = KT_tile.T @ QT_block -> exp on ScalarE -> PT
     bf16; out[q, d|sum] = PT.T @ [V|1] (softmax denominator rides along as a
     129th matmul column); rows normalized by its reciprocal, then transposed
     on the TensorE so the AllToAll lands in [d_concat, q] (lhsT) layout.
  3. After each A2A piece the core holds AT_s[2048, q_s] for its own output
     rows; plain chunk loads feed out = A @ Wo.
"""

import os
import sys
import types

import numpy as np
import ml_dtypes

import concourse.bass as bass
import concourse.mybir as mybir
import concourse.tile as tile
from concourse import bacc
from concourse.bass_utils import run_bass_kernel_spmd
from concourse.masks import make_identity
from concourse.tile_rust import add_dep_helper

BF16 = mybir.dt.bfloat16
F32 = mybir.dt.float32
nbf16 = ml_dtypes.bfloat16

N_CORES = 8
L = 4096
D = 2048
HPC = 2  # heads per core
HD = 128  # head dim
FC = HPC * HD  # 256: per-core projection width
KCH = D // 128  # 16 contraction chunks
SCALE = 1.0 / float(np.sqrt(HD))
QB = 512  # attention q block
NKK = L // 128  # 32 key tiles

# A2A splits: (q0, qlen, block_q). Output rows of core c:
#   s0 -> global [c*256, +256),  s1 -> [2048+c*128, +128),  s2 -> [3072+c*128, +128)
SPLITS = [(0, 2048, 256), (2048, 1024, 128), (3072, 1024, 128)]
OUT_ROW0 = [0, 256, 384]

# module-level knobs (test.py pokes these)
TRACE = False
LAST_RESULTS = None
_CACHED = {}


def _patch_walrus_flags():
    """Enable the LDWEIGHTS fast path (fast weight load) in walrus."""
    from concourse import bass_utils as _bu

    if getattr(_bu, "_ldw_patched", False):
        return
    _orig = _bu.run_command

    def _patched(cmd, **kw):
        cmd = [
            c
            for c in cmd
        ]
        return _orig(cmd, **kw)

    _bu.run_command = _patched
    _bu._ldw_patched = True


def _install_ntff_hook():
    """Enable NTFF profiling under axon (the container lacks antenv.axon_hooks)."""
    try:
        if "antenv.axon_hooks" not in sys.modules:
            mod = types.ModuleType("antenv.axon_hooks")
            _hook = [None]
            mod.set_axon_ntff_profile_hook = lambda h: _hook.__setitem__(0, h)
            mod.get_axon_ntff_profile_hook = lambda: _hook[0]
            sys.modules["antenv.axon_hooks"] = mod
            import antenv

            antenv.axon_hooks = mod
        from antenv.axon_hooks import set_axon_ntff_profile_hook
        from trn_agent_boot.trn_boot import _ntff_profile_via_ctypes

        set_axon_ntff_profile_hook(_ntff_profile_via_ctypes("/opt/axon/libaxon_pjrt.so"))
        from concourse import bass_utils

        bass_utils.upload_artifacts = lambda tmpdir: tmpdir
    except Exception:
        pass


def build_nc():
    nc = bacc.Bacc(None, target_bir_lowering=False, num_devices=N_CORES)

    xT_ext = nc.declare_dram_parameter("xT", [D, L], BF16, isOutput=False)
    wq_ext = nc.declare_dram_parameter("wq", [D, FC], BF16, isOutput=False)
    wk_ext = nc.declare_dram_parameter("wk", [D, FC], BF16, isOutput=False)
    wv_ext = nc.declare_dram_parameter("wv", [D, FC], BF16, isOutput=False)
    wo_ext = nc.declare_dram_parameter("wo", [D, D], BF16, isOutput=False)
    ctab_ext = nc.declare_dram_parameter("ctab", [128, L], BF16, isOutput=False)
    stab_ext = nc.declare_dram_parameter("stab", [128, L], BF16, isOutput=False)
    out_ext = nc.declare_dram_parameter("out", [512, D], F32, isOutput=True)

    # A2A bounces in [d_concat-block, q] layout: rows = 8 blocks x (2 heads x 128 d)
    a2a_in = [
        nc.dram_tensor(f"a2a_in{s}", [8 * FC, bq], BF16)
        for s, (_, _, bq) in enumerate(SPLITS)
    ]
    a2a_out = [
        nc.dram_tensor(f"a2a_out{s}", [8 * FC, bq], BF16)
        for s, (_, _, bq) in enumerate(SPLITS)
    ]

    sync_in = nc.dram_tensor("sync_in", [8, 64], BF16)
    sync_out = nc.dram_tensor("sync_out", [8, 64], BF16)

    with tile.TileContext(nc) as tc:
        with tc.tile_pool(name="persist", bufs=1) as persist:
            # barrier-warming collective: absorbs per-core start skew so the
            # first real AllToAll doesn't pay it
            nc.gpsimd.collective_compute(
                "AllToAll",
                mybir.AluOpType.bypass,
                replica_groups=[list(range(N_CORES))],
                ins=[sync_in.ap().opt()],
                outs=[sync_out.ap().opt()],
            )
            # persistent tiles (no DMA yet; emission order sets DMA priority)
            qt = persist.tile([128, HPC * L], BF16, tag="qt")
            kt = persist.tile([128, HPC * L], BF16, tag="kt")
            va = persist.tile([128, (L // 128) * 260], BF16, tag="va")
            wo_sb = persist.tile([128, KCH * D], BF16, tag="wo")
            warm = persist.tile([128, 8], BF16, tag="warm")

            QL = 1024  # L columns per xT load round
            with (
                tc.tile_pool(name="p1sb", bufs=1) as p1,
                tc.tile_pool(name="p1ps", bufs=1, space="PSUM") as p1ps,
            ):
                # critical-path loads first: wq, wk, xT quarter 0
                ctab = p1.tile([128, L], BF16, tag="ctab")
                stab = p1.tile([128, L], BF16, tag="stab")
                wq_sb = p1.tile([128, KCH * FC], BF16, tag="wq")
                wk_sb = p1.tile([128, KCH * FC], BF16, tag="wk")
                wv_sb = p1.tile([128, KCH * FC], BF16, tag="wv")
                # per-kc interleaved loads so the first Q-proj matmul can
                # start after ~2 chunks instead of after the full weights
                xts_q0 = []
                for kc in range(KCH):
                    nc.gpsimd.dma_start(
                        wq_sb[:, kc * FC : (kc + 1) * FC],
                        wq_ext[kc * 128 : (kc + 1) * 128, :],
                    )
                    nc.sync.dma_start(
                        wk_sb[:, kc * FC : (kc + 1) * FC],
                        wk_ext[kc * 128 : (kc + 1) * 128, :],
                    )
                    xt_t = p1.tile([128, QL], BF16, tag="xt", bufs=18)
                    eng = (nc.gpsimd, nc.sync)[kc % 2]
                    eng.dma_start(xt_t[:], xT_ext[kc * 128 : (kc + 1) * 128, 0:QL])
                    xts_q0.append(xt_t)
                nc.gpsimd.dma_start(ctab[:], ctab_ext[:])
                nc.gpsimd.dma_start(stab[:], stab_ext[:])
                nc.gpsimd.dma_start(
                    wv_sb[:].rearrange("p (k f) -> p k f", k=KCH),
                    wv_ext[:].rearrange("(k p) f -> p k f", p=128),
                )
                nc.gpsimd.memset(va[:], 1.0)
                nc.scalar.activation(
                    warm[:], ctab[:, 0:8], mybir.ActivationFunctionType.Exp
                )
                nc.gpsimd.dma_start(
                    wo_sb[:].rearrange("p (k f) -> p k f", k=KCH),
                    wo_ext[:].rearrange("(k p) f -> p k f", p=128),
                )

                # ---------------- Phase 1: QKV projections + RoPE ------------
                for quarter in range(L // QL):
                    l0 = quarter * QL
                    if quarter == 0:
                        xts = xts_q0
                    else:
                        xts = []
                        for kc in range(KCH):
                            xt_t = p1.tile([128, QL], BF16, tag="xt", bufs=18)
                            nc.gpsimd.dma_start(
                                xt_t[:], xT_ext[kc * 128 : (kc + 1) * 128, l0 : l0 + QL]
                            )
                            xts.append(xt_t)
                    # Q and K projections (transposed layout) + rope
                    for (w_sb, dst) in ((wq_sb, qt), (wk_sb, kt)):
                        for h in range(HPC):
                            for lb in range(QL // 512):
                                ps = p1ps.tile([128, 512], F32, tag="pj", bufs=4)
                                for kc in range(KCH):
                                    nc.tensor.matmul(
                                        ps[:],
                                        w_sb[:, kc * FC + h * HD : kc * FC + (h + 1) * HD],
                                        xts[kc][:, lb * 512 : (lb + 1) * 512],
                                        start=(kc == 0),
                                        stop=(kc == KCH - 1),
                                    )
                                lsl = slice(l0 + lb * 512, l0 + (lb + 1) * 512)
                                tmp = p1.tile([128, 512], BF16, tag="tmp", bufs=4)
                                nc.scalar.copy(tmp[:], ps[:])
                                rot = p1.tile([128, 512], BF16, tag="rot", bufs=4)
                                for (a, b) in ((0, 64), (32, 96), (64, 0), (96, 32)):
                                    nc.vector.tensor_copy(
                                        rot[a : a + 32, :], tmp[b : b + 32, :]
                                    )
                                t1 = p1.tile([128, 512], BF16, tag="t1", bufs=4)
                                nc.vector.tensor_mul(t1[:], tmp[:], ctab[:, lsl])
                                t2 = p1.tile([128, 512], BF16, tag="t2", bufs=4)
                                nc.vector.tensor_mul(t2[:], rot[:], stab[:, lsl])
                                dsl = slice(
                                    h * L + l0 + lb * 512, h * L + l0 + (lb + 1) * 512
                                )
                                nc.vector.tensor_add(dst[:, dsl], t1[:], t2[:])
                    # V projection (natural layout), strided copy into va
                    for lt in range(QL // 128):
                        psv = p1ps.tile([128, FC], F32, tag="pv", bufs=3)
                        for kc in range(KCH):
                            nc.tensor.matmul(
                                psv[:],
                                xts[kc][:, lt * 128 : (lt + 1) * 128],
                                wv_sb[:, kc * FC : (kc + 1) * FC],
                                start=(kc == 0),
                                stop=(kc == KCH - 1),
                            )
                        gt = quarter * (QL // 128) + lt  # global L tile 0..31
                        dst = va[:, gt * 260 : (gt + 1) * 260].rearrange(
                            "p (g j) -> p g j", g=2
                        )[:, :, 0:128]
                        nc.vector.tensor_copy(
                            dst, psv[:].rearrange("p (g j) -> p g j", g=2)
                        )

            # -------- Phase 2: attention windows + overlapped A2A/Wo ---------
            windows = [(qb, h) for qb in range(L // QB) for h in range(HPC)]
            with (
                tc.tile_pool(name="p2sb", bufs=1) as p2,
                tc.tile_pool(name="p2ps", bufs=1, space="PSUM") as p2ps,
                tc.tile_pool(name="p3sb", bufs=1) as p3,
                tc.tile_pool(name="p3ps", bufs=1, space="PSUM") as p3ps,
            ):
                pt_store = {}

                def emit_epilogue(w, qs, av):
                    qb, h = windows[w]
                    rec = p2.tile([128, 1], F32, tag="rec", bufs=12)
                    nc.vector.reciprocal(rec[:], av[:, 128:129])
                    osb = p2.tile([128, 128], BF16, tag="osb", bufs=12)
                    nc.vector.tensor_scalar_mul(osb[:], av[:, 0:128], rec[:])
                    # xbar-transpose on the sync DMA queue so the A2A
                    # carries [d, q]-layout tiles
                    ot = p2.tile([128, 128], BF16, tag="ot", bufs=12)
                    nc.sync.dma_start_transpose(ot[:], osb[:])
                    t = qb * (QB // 128) + qs  # global q tile 0..31
                    qg = t * 128
                    s = next(
                        i for i, (q0, ql, _) in enumerate(SPLITS) if q0 <= qg < q0 + ql
                    )
                    q0, _, bq = SPLITS[s]
                    tt = (qg - q0) // 128
                    tpb = bq // 128
                    j, co = tt // tpb, (tt % tpb) * 128
                    nc.gpsimd.dma_start(
                        a2a_in[s][j * FC + h * HD : j * FC + (h + 1) * HD, co : co + 128],
                        ot[:],
                    )

                def emit_window(w):
                    # scores+exp for window w interleaved with AV for w-1, so
                    # the PE never idles waiting on the exp pipeline
                    prev = pt_store.pop(w - 1, None)
                    pts = []
                    av = None
                    last_av_mm = None
                    for p in range(NKK // 2):  # 16 steps
                        if w < len(windows):
                            qb, h = windows[w]
                            sc = p2ps.tile([128, 1024], F32, tag="sc", bufs=2)
                            for half in range(2):
                                kk = p * 2 + half
                                nc.tensor.matmul(
                                    sc[:, half * 512 : (half + 1) * 512],
                                    kt[:, h * L + kk * 128 : h * L + (kk + 1) * 128],
                                    qt[:, h * L + qb * QB : h * L + (qb + 1) * QB],
                                )
                            pt = p2.tile([128, 1024], BF16, tag="pt", bufs=32)
                            nc.scalar.activation(
                                pt[:], sc[:], mybir.ActivationFunctionType.Exp, scale=SCALE
                            )
                            pts.append(pt)
                        if prev is not None:
                            pqb, ph = windows[w - 1]
                            qs = p // 4
                            if p % 4 == 0:
                                av = p2ps.tile([128, 129], F32, tag="av", bufs=3)
                            for kk in range((p % 4) * 8, (p % 4) * 8 + 8):
                                last_av_mm = nc.tensor.matmul(
                                    av[:],
                                    prev[kk // 2][
                                        :,
                                        (kk % 2) * 512 + qs * 128 : (kk % 2) * 512 + (qs + 1) * 128,
                                    ],
                                    va[:, kk * 260 + ph * 130 : kk * 260 + ph * 130 + 129],
                                    start=(kk == 0),
                                    stop=(kk == NKK - 1),
                                    skip_group_check=True,
                                )
                            if p % 4 == 3:
                                emit_epilogue(w - 1, qs, av)
                    if pts:
                        pt_store[w] = pts
                    return last_av_mm

                ats = {}

                def emit_wo_comm(s):
                    nc.gpsimd.collective_compute(
                        "AllToAll",
                        mybir.AluOpType.bypass,
                        replica_groups=[list(range(N_CORES))],
                        ins=[a2a_in[s].ap().opt()],
                        outs=[a2a_out[s].ap().opt()],
                    )

                def emit_at_load(s, engines):
                    # split the 1MB load across two DMA rings (one ring
                    # moves only ~50GB/s for this descriptor shape); the
                    # triggers are emitted late enough that the A2A-done
                    # semaphore has already fired, so no head-of-line block
                    bq = SPLITS[s][2]
                    at = p3.tile([128, KCH * 256], BF16, tag="at", bufs=2)
                    h = KCH // 2
                    for i, eng in enumerate(engines):
                        eng.dma_start(
                            at[:, i * h * bq : (i + 1) * h * bq].rearrange(
                                "p (k q) -> p k q", k=h
                            ),
                            a2a_out[s][i * h * 128 : (i + 1) * h * 128, :].rearrange(
                                "(k p) q -> p k q", p=128
                            ),
                        )
                    ats[s] = at

                def emit_wo_chunk(s, g, after=None):
                    bq = SPLITS[s][2]
                    rt, fb = g // 4, g % 4
                    po = p3ps.tile([128, 512], F32, tag="po", bufs=1)
                    for kc in range(KCH):
                        mm = nc.tensor.matmul(
                            po[:],
                            ats[s][:, kc * bq + rt * 128 : kc * bq + (rt + 1) * 128],
                            wo_sb[:, kc * D + fb * 512 : kc * D + (fb + 1) * 512],
                            start=(kc == 0),
                            stop=(kc == KCH - 1),
                            skip_group_check=True,
                        )
                        if kc == 0 and after is not None:
                            add_dep_helper(
                                mm.ins,
                                after.ins,
                                sync=False,
                                reason="keep Wo chunk behind the attention window",
                            )
                    ob = p3.tile([128, 512], F32, tag="ob", bufs=2)
                    # DVE, not ACT: ACT is saturated with exp during the windows,
                    # and the single-buffered po bank recycles through this copy
                    nc.vector.tensor_copy(ob[:], po[:])
                    r0 = OUT_ROW0[s] + rt * 128
                    nc.sync.dma_start(
                        out_ext[r0 : r0 + 128, fb * 512 : (fb + 1) * 512], ob[:]
                    )

                for w in range(len(windows) + 1):
                    if w == 13:
                        # A2A s0 completed ~40us ago; trigger the AT load
                        # from the PE + sync queues (both reach this point
                        # well after the semaphore fired)
                        emit_at_load(0, engines=(nc.sync, nc.gpsimd))
                    if w == 15:
                        emit_at_load(1, engines=(nc.sync, nc.gpsimd))
                    la = emit_window(w)
                    if w == 8:
                        # split-0 tiles (q < 2048, both heads) stored after
                        # emit_av(7) inside this same iteration
                        emit_wo_comm(0)
                    if w == 12:
                        # split-1 tiles (2048 <= q < 3072) stored after
                        # emit_av(11) inside this same iteration
                        emit_wo_comm(1)
                    # only 2 Wo chunks per window (PE slack vs the exp
                    # cadence); the rest are held back to keep the PE fed
                    # while the final A2A + AT load are in flight
                    if 14 <= w <= 16:
                        emit_wo_chunk(0, 2 * (w - 14), after=la)
                        emit_wo_chunk(0, 2 * (w - 14) + 1, after=la)
                emit_wo_comm(2)
                for s, g in ((0, 6), (0, 7), (1, 0), (1, 1), (1, 2), (1, 3)):
                    emit_wo_chunk(s, g)
                emit_at_load(2, engines=(nc.sync, nc.gpsimd))
                for g in range(4):
                    emit_wo_chunk(2, g)

    nc.compile()
    return nc


def _host_prep(x, Wq, Wk, Wv, Wo, sin, cos):
    xT = np.ascontiguousarray(x.T).astype(nbf16)
    wo_b = np.ascontiguousarray(Wo).astype(nbf16)
    c64 = cos.reshape(L, 64)
    s64 = sin.reshape(L, 64)
    ctab = np.ascontiguousarray(np.concatenate([c64, c64], axis=1).T).astype(nbf16)
    stab = np.ascontiguousarray(np.concatenate([-s64, s64], axis=1).T).astype(nbf16)
    in_maps = []
    for c in range(N_CORES):
        sl = slice(c * FC, (c + 1) * FC)
        in_maps.append(
            {
                "xT": xT,
                "wq": np.ascontiguousarray(Wq[:, sl]).astype(nbf16),
                "wk": np.ascontiguousarray(Wk[:, sl]).astype(nbf16),
                "wv": np.ascontiguousarray(Wv[:, sl]).astype(nbf16),
                "wo": wo_b,
                "ctab": ctab,
                "stab": stab,
            }
        )
    return in_maps


def kernel(x, Wq, Wk, Wv, Wo, sin, cos):
    global LAST_RESULTS
    x, Wq, Wk, Wv, Wo = (np.asarray(a, np.float32) for a in (x, Wq, Wk, Wv, Wo))
    sin, cos = np.asarray(sin, np.float32), np.asarray(cos, np.float32)

    _patch_walrus_flags()
    if TRACE:
        _install_ntff_hook()
        os.environ["BASS_TRACE"] = "1"

    if "nc" not in _CACHED:
        _CACHED["nc"] = build_nc()
    nc = _CACHED["nc"]

    in_maps = _host_prep(x, Wq, Wk, Wv, Wo, sin, cos)
    trace_cores = list(range(N_CORES)) if os.environ.get("ALL_CORES") else None
    res = run_bass_kernel_spmd(
        nc, in_maps, core_ids=list(range(N_CORES)), trace=TRACE, trace_cores=trace_cores
    )
    LAST_RESULTS = res

    out = np.empty((L, D), np.float32)
    for c in range(N_CORES):
        oc = res.results[c]["out"]
        out[c * 256 : (c + 1) * 256] = oc[0:256]
        out[2048 + c * 128 : 2048 + (c + 1) * 128] = oc[256:384]
        out[3072 + c * 128 : 3072 + (c + 1) * 128] = oc[384:512]
    return out

